# revision 9
# baseline (speedup 1.0000x reference)
"""Multi-head attention (shared key head) on 8 TRN2 NeuronCores.

Sharding: core c handles batch b = c % 4 and head group g = c // 4
(heads 4g..4g+3).  Per-core weights are sliced on host; x is
pre-transposed (and bf16-cast) on host so the device never transposes
the activations.

Device-side per core (bf16 matmul path, fp32 PSUM accumulation):
  xT [512, 2048]  ->  QT [a, s] (2 tiles of 128 part), KT [64, s] dup'd
                      into both partition halves, V [s, 4*(64+1)]
                      (per-head 65-col block: V+bv, ones col)
  scores^T[k, q] = KT_tile^T . QT_head   (k on partitions, q moving)
  attn^T = exp(scale * scores^T)         (no max subtraction: |scores|<~0.3)
  causal: only k-tiles inside the 1024-wide q-chunk's causal extent are
  computed, diagonal tiles restrict the moving range, one triangular
  mask multiply on the 128-wide boundary block.
  out^T[o(+denom), q] accumulates V_aug^T . attn^T in PSUM; the ones
  column of V_aug yields the softmax denominator in row 64.
  Epilogue: PE-transpose 128-col chunks, reciprocal of denom column,
  per-partition scalar multiply, DMA out.
"""

import math
import numpy as np
import ml_dtypes

import concourse.bass as bass
import concourse.mybir as mybir
import concourse.tile as tile
from concourse import bacc
from concourse.bass_utils import run_bass_kernel_spmd

B, S, D = 4, 2048, 512
H, A, O = 8, 64, 64
NCORES = 8
HPC = 4                # heads per core
APC = HPC * A          # 256 projection cols per core
VBLK = O + 1           # per-head V block width (out cols + ones col)
SCALE = 1.0 / math.sqrt(S)

F32 = mybir.dt.float32
BF16 = mybir.dt.bfloat16
AF = mybir.ActivationFunctionType
BF_NP = ml_dtypes.bfloat16

QC = 1024              # attention q-chunk width
N_QC = S // QC         # 2
N_DT = D // 128        # 4 contraction tiles
N_SC = S // 512        # 4 s-chunks of 512
N_ST = S // 128        # 16 s-tiles / k-tiles of 128


def build():
    nc = bacc.Bacc("TRN2", target_bir_lowering=False, debug=False,
                   num_devices=NCORES)

    xT_d = nc.dram_tensor("xT", [D, S], BF16, kind="ExternalInput").ap()
    wq_d = nc.dram_tensor("wq", [D, APC], BF16, kind="ExternalInput").ap()
    bq_d = nc.dram_tensor("bq", [2, 128, 1], F32, kind="ExternalInput").ap()
    wk_d = nc.dram_tensor("wk", [D, A], BF16, kind="ExternalInput").ap()
    wv_d = nc.dram_tensor("wv", [D, APC], BF16, kind="ExternalInput").ap()
    bvm_d = nc.dram_tensor("bvm", [128, HPC * VBLK], BF16,
                           kind="ExternalInput").ap()
    out_d = nc.dram_tensor("out", [S, APC], F32, kind="ExternalOutput").ap()

    tri_np = (np.arange(128)[None, :] >= np.arange(128)[:, None])
    tri_d = nc.inline_tensor(tri_np.astype(BF_NP), "tri").ap()
    eye_d = nc.inline_tensor(np.eye(128, dtype=np.float32), "eye").ap()

    with tile.TileContext(nc) as tc:
        with tc.tile_pool(name="const", bufs=1) as cpool, \
             tc.tile_pool(name="persist", bufs=1) as ppool, \
             tc.tile_pool(name="attn", bufs=6) as apool, \
             tc.tile_pool(name="outt", bufs=3) as opool, \
             tc.tile_pool(name="fin", bufs=6) as fpool, \
             tc.tile_pool(name="ps_sc", bufs=2, space="PSUM") as ps_sc, \
             tc.tile_pool(name="ps_av", bufs=1, space="PSUM") as ps_av:

            # ---- constants / weights to SBUF ----
            tri = cpool.tile([128, 128], BF16, tag="tri", name="tri")
            eye = cpool.tile([128, 128], F32, tag="eye", name="eye")
            bvm = cpool.tile([128, HPC * VBLK], BF16, tag="bvm", name="bvm")
            nc.sync.dma_start(out=tri[:, :], in_=tri_d[:, :])
            nc.sync.dma_start(out=eye[:, :], in_=eye_d[:, :])
            nc.sync.dma_start(out=bvm[:, :], in_=bvm_d[:, :])

            wq_sb, wk_sb, wv_sb = [], [], []
            for dt in range(N_DT):
                wq_t = cpool.tile([128, APC], BF16, tag=f"wq{dt}", name=f"wq{dt}")
                wk_t = cpool.tile([128, A], BF16, tag=f"wk{dt}", name=f"wk{dt}")
                wv_t = cpool.tile([128, APC], BF16, tag=f"wv{dt}", name=f"wv{dt}")
                r = slice(dt * 128, (dt + 1) * 128)
                nc.sync.dma_start(out=wq_t[:, :], in_=wq_d[r, :])
                nc.sync.dma_start(out=wk_t[:, :], in_=wk_d[r, :])
                nc.sync.dma_start(out=wv_t[:, :], in_=wv_d[r, :])
                wq_sb.append(wq_t)
                wk_sb.append(wk_t)
                wv_sb.append(wv_t)

            bq_sb = []
            for at in range(2):
                t = cpool.tile([128, 1], F32, tag=f"bq{at}", name=f"bq{at}")
                nc.sync.dma_start(out=t[:, :], in_=bq_d[at])
                bq_sb.append(t)

            # ---- x^T to SBUF ----
            xt = []
            for dt in range(N_DT):
                t = ppool.tile([128, S], BF16, tag=f"xt{dt}", name=f"xt{dt}")
                for sc in range(N_SC):
                    cs = slice(sc * 512, (sc + 1) * 512)
                    nc.sync.dma_start(out=t[:, cs],
                                      in_=xT_d[dt * 128:(dt + 1) * 128, cs])
                xt.append(t)

            # ---- projections ----
            # QT: [a, s] packed 2 heads per 128-partition tile
            qt = [ppool.tile([128, S], BF16, tag=f"qt{at}", name=f"qt{at}")
                  for at in range(2)]
            for at in range(2):
                for sc in range(N_SC):
                    cs = slice(sc * 512, (sc + 1) * 512)
                    ps = ps_sc.tile([128, 512], F32, tag="sc", name="sc")
                    for dt in range(N_DT):
                        nc.tensor.matmul(
                            out=ps[:, :],
                            lhsT=wq_sb[dt][:, at * 128:(at + 1) * 128],
                            rhs=xt[dt][:, cs],
                            start=(dt == 0), stop=(dt == N_DT - 1))
                    nc.vector.tensor_scalar_add(out=qt[at][:, cs],
                                                in0=ps[:, :],
                                                scalar1=bq_sb[at][:, :])

            # KT duplicated into both partition halves so the scores matmul
            # lhsT base partition matches the QT head slice (0 or 64).
            kt = ppool.tile([128, S], BF16, tag="kt", name="kt")
            for sc in range(N_SC):
                cs = slice(sc * 512, (sc + 1) * 512)
                ps = ps_sc.tile([64, 512], F32, tag="sc", name="sc")
                for dt in range(N_DT):
                    nc.tensor.matmul(out=ps[:, :], lhsT=wk_sb[dt][:, :],
                                     rhs=xt[dt][:, cs],
                                     start=(dt == 0), stop=(dt == N_DT - 1))
                nc.vector.tensor_copy(kt[0:64, cs], ps[:, :])
                nc.vector.tensor_copy(kt[64:128, cs], ps[:, :])

            # V: per s-tile [128, 4*65]; 65th col of each head block = 1.0
            vt = []
            for st in range(N_ST):
                t = ppool.tile([128, HPC * VBLK], BF16, tag=f"v{st}",
                               name=f"v{st}")
                nc.vector.tensor_copy(t[:, O:HPC * VBLK:VBLK],
                                      bvm[:, O:HPC * VBLK:VBLK])
                vt.append(t)
            for st in range(N_ST):
                ps = ps_sc.tile([128, APC], F32, tag="sc", name="sc")
                for dt in range(N_DT):
                    nc.tensor.matmul(
                        out=ps[:, :],
                        lhsT=xt[dt][:, st * 128:(st + 1) * 128],
                        rhs=wv_sb[dt][:, :],
                        start=(dt == 0), stop=(dt == N_DT - 1))
                for h in range(HPC):
                    nc.vector.tensor_add(
                        out=vt[st][:, h * VBLK:h * VBLK + O],
                        in0=ps[:, h * O:(h + 1) * O],
                        in1=bvm[:, h * VBLK:h * VBLK + O])

            # ---- attention ----
            for h in range(HPC):
                at, poff = h // 2, (h % 2) * 64
                for qc in range(N_QC):
                    av = ps_av.tile([128, QC], F32, tag="av", name="av")
                    nkj = (QC // 128) * (qc + 1)
                    for kj in range(nkj):
                        m = kj - (QC // 128) * qc
                        vs = 128 * m if m > 0 else 0     # valid q start
                        qlo = qc * QC
                        sc_ps = ps_sc.tile([128, QC], F32, tag="sc", name="sc")
                        for hf in range(QC // 512):
                            lo = max(vs, hf * 512)
                            hi = (hf + 1) * 512
                            if lo >= hi:
                                continue
                            nc.tensor.matmul(
                                out=sc_ps[:, lo:hi],
                                lhsT=kt[poff:poff + 64,
                                        kj * 128:(kj + 1) * 128],
                                rhs=qt[at][poff:poff + 64, qlo + lo:qlo + hi],
                                start=True, stop=True)
                        atn = apool.tile([128, QC], BF16, tag="atn", name="atn")
                        nc.scalar.activation(out=atn[:, vs:QC],
                                             in_=sc_ps[:, vs:QC],
                                             func=AF.Exp, scale=SCALE)
                        if m >= 0:
                            nc.vector.tensor_mul(out=atn[:, vs:vs + 128],
                                                 in0=atn[:, vs:vs + 128],
                                                 in1=tri[:, :])
                        for hf in range(QC // 512):
                            lo = max(vs, hf * 512)
                            hi = (hf + 1) * 512
                            if lo >= hi:
                                continue
                            # last k-tile whose valid q-range still reaches
                            # this 512-half closes that bank's accum group
                            last_kj = nkj - 1 if hf == 1 else \
                                (QC // 128) * qc + 3
                            nc.tensor.matmul(
                                out=av[0:VBLK, lo:hi],
                                lhsT=vt[kj][:, h * VBLK:(h + 1) * VBLK],
                                rhs=atn[:, lo:hi],
                                start=(kj == 0), stop=(kj == last_kj))

                    # epilogue for this (head, q-chunk)
                    ot = opool.tile([VBLK, QC], F32, tag="ot", name="ot")
                    nc.vector.tensor_copy(ot[:, :], av[0:VBLK, :])
                    for j in range(QC // 128):
                        ep = ps_sc.tile([128, VBLK], F32, tag="ep", name="ep")
                        nc.tensor.transpose(
                            out=ep[:, :],
                            in_=ot[:, j * 128:(j + 1) * 128],
                            identity=eye[0:VBLK, 0:VBLK])
                        rc = fpool.tile([128, 1], F32, tag="rc", name="rc")
                        nc.vector.reciprocal(rc[:, :], ep[:, O:O + 1])
                        fo = fpool.tile([128, O], F32, tag="fo", name="fo")
                        nc.vector.tensor_scalar_mul(out=fo[:, :],
                                                    in0=ep[:, 0:O],
                                                    scalar1=rc[:, :])
                        q0 = qc * QC + j * 128
                        nc.sync.dma_start(
                            out=out_d[q0:q0 + 128, h * O:(h + 1) * O],
                            in_=fo[:, :])

    nc.compile()
    return nc


_NC = None
LAST_RESULTS = None


def _bvm(bv_slice):
    blk = np.empty((HPC, VBLK), dtype=np.float32)
    blk[:, :O] = np.asarray(bv_slice, dtype=np.float32).reshape(HPC, O)
    blk[:, O] = 1.0
    return np.ascontiguousarray(np.broadcast_to(
        blk.reshape(1, HPC * VBLK), (128, HPC * VBLK))).astype(BF_NP)


def make_in_maps(x, Wq, bq, Wk, Wv, bv):
    in_maps = []
    for c in range(NCORES):
        b, g = c % 4, c // 4
        cols = slice(g * APC, (g + 1) * APC)
        in_maps.append({
            "xT": np.ascontiguousarray(x[b].T).astype(BF_NP),
            "wq": np.ascontiguousarray(Wq[:, cols]).astype(BF_NP),
            "bq": np.ascontiguousarray(bq[cols].reshape(2, 128, 1)),
            "wk": np.ascontiguousarray(Wk).astype(BF_NP),
            "wv": np.ascontiguousarray(Wv[:, cols]).astype(BF_NP),
            "bvm": _bvm(bv[cols]),
        })
    return in_maps


def kernel(**inputs):
    global _NC, LAST_RESULTS
    x = np.asarray(inputs["x"], dtype=np.float32)
    Wq = np.asarray(inputs["Wq"], dtype=np.float32)
    bq = np.asarray(inputs["bq"], dtype=np.float32)
    Wk = np.asarray(inputs["Wk"], dtype=np.float32)
    Wv = np.asarray(inputs["Wv"], dtype=np.float32)
    bv = np.asarray(inputs["bv"], dtype=np.float32)

    if _NC is None:
        _NC = build()

    in_maps = make_in_maps(x, Wq, bq, Wk, Wv, bv)
    res = run_bass_kernel_spmd(_NC, in_maps, core_ids=list(range(NCORES)))
    LAST_RESULTS = res

    out = np.empty((B, S, H * O), dtype=np.float32)
    for c in range(NCORES):
        b, g = c % 4, c // 4
        out[b, :, g * APC:(g + 1) * APC] = res.results[c]["out"]
    return out
